# revision 1
# baseline (speedup 1.0000x reference)
"""MBD degradation-imputation sampling step on 8 Trainium2 NeuronCores.

Strategy (data-parallel over the N=2048 candidate samples, 256/core):
  pass A : per-sample consistency scores, ONE HBM pass over eps.
           Block-major (two 128-sample blocks) so block 0's score
           AllGather hides under block 1's compute.  Per [128,1024]
           tile, all on the Vector engine (measured: STT=1.0c/e,
           TS=0.5c/e; TENSOR_TENSOR is 2.5c/e and Pool ops are >2.4c/e
           and contend for SBUF — avoid both):
               u  = eps + c0          (DVE STT in-place on the eps tile)
               vh = clip(u, +-1/s)    (DVE TS -> fp16 CACHE in SBUF)
               Sb += sum(q'*vh)       (DVE STT + accum_out)
               Sa += sum(vh^2)        (ACT Square + accum_out)
           score = -s^2/TF*(Sa - 2*Sb) (+ sample-independent shift;
           observed positions saturate via c0=1e6 so they cancel).
           The fp16 clipped values stay resident in SBUF (128 KiB/pn)
           so pass B never re-reads eps (saves 32 MB/core of HBM).
  softmax: stats on the gathered 2048 scores; (1/(std*T), -mx/(std*T))
           broadcast to 128 partitions via a K=1 PE matmul; weights are
           UN-normalized exp(); the local normalizer Z rides slot TF of
           the AllReduce buffer and division happens post-reduce.
  pass B : weighted partition-reduction straight out of the fp16 SBUF
           cache on the TensorEngine (fp16 matmuls, M=1, PSUM-
           accumulated), PSUM->SBUF bounce split Vector/Scalar,
           AllReduce the (T,F)+Z partials (128 KB), final
           out = mask ? obs : c1*sigma/Z * weighted.

`stage` truncates the program for hardware bisection:
  1 = pass A only, 2 = +AllGather/softmax, 3 = +pass B (no AllReduce),
  4 = full kernel.
"""

from contextlib import ExitStack

import numpy as np

import concourse.bass as bass
import concourse.tile as tile
from concourse import bacc, mybir
from concourse.bass_utils import run_bass_kernel_spmd

N_CORES = 8
N, T, F = 2048, 512, 64
P = 128
TF = T * F                      # 32768
NLOC = N // N_CORES             # 256
NBLK = NLOC // P                # 2
CHUNK = 1024
NCHUNK = TF // CHUNK            # 32
SUB = 512                       # matmul N (one PSUM bank)
TEMP = 0.1
T_STEPS = 1000

F32 = mybir.dt.float32
F16 = mybir.dt.float16
AX = mybir.AxisListType
ALU = mybir.AluOpType
ACTF = mybir.ActivationFunctionType


def _schedule_scalars(i: int):
    s = 0.008
    x = np.linspace(0, T_STEPS, T_STEPS + 1, dtype=np.float64)
    ac = np.cos((x / T_STEPS + s) / (1 + s) * np.pi * 0.5) ** 2
    ac = ac / ac[0]
    betas = np.clip(1.0 - ac[1:] / ac[:-1], 0.0, 0.999)
    alphas = 1.0 - betas
    acp = np.cumprod(alphas)
    abar_i = np.float32(acp[i])
    sigma_i = np.float32(np.sqrt(1.0 - acp[i]))
    alpha_i = np.float32(alphas[i])
    abar_im1 = np.float32(acp[i - 1])
    sa = np.float32(np.sqrt(abar_i))
    # the reference's Yi terms cancel exactly; out_missing = c1 * weighted
    c1 = np.float32(sa / np.float32(np.sqrt(alpha_i)) / np.float32(np.sqrt(abar_im1)))
    return sigma_i, c1


def _build(sigma_i: float, c1: float, stage: int = 4):
    inv_sig = float(np.float32(1.0 / np.float32(sigma_i)))
    sigma_i = float(np.float32(sigma_i))
    c1 = float(np.float32(c1))
    # scores = cA * sum((v - q')^2)  (+ sample-independent shift vs ref)
    cA = float(np.float32(-(np.float32(sigma_i) ** 2) / np.float32(TF)))

    nc = bacc.Bacc(
        "TRN2", target_bir_lowering=False, debug=False, num_devices=N_CORES
    )
    eps_d = nc.dram_tensor("eps", [NLOC, TF], F32, kind="ExternalInput")
    c0_d = nc.dram_tensor("c0", [TF], F32, kind="ExternalInput")
    qp_d = nc.dram_tensor("qp", [TF], F16, kind="ExternalInput")
    obs_d = nc.dram_tensor("obs", [TF], F32, kind="ExternalInput")
    maskf_d = nc.dram_tensor("maskf", [TF], F32, kind="ExternalInput")
    out_d = nc.dram_tensor("out", [TF], F32, kind="ExternalOutput")

    ones_d = nc.dram_tensor("ones", [P], F32, kind="ExternalInput")
    sc_loc_d = nc.dram_tensor("sc_loc", [NLOC], F32)
    sc_all_d = nc.dram_tensor("sc_all", [N], F32, addr_space="Shared")
    # ws carries the TF weighted partials plus the local softmax
    # normalizer Z in slot TF — one AllReduce delivers both.
    ws_loc_d = nc.dram_tensor("ws_loc", [TF + 4], F32)
    ws_all_d = nc.dram_tensor("ws_all", [TF + 4], F32, addr_space="Shared")

    rg = [list(range(N_CORES))]

    with tile.TileContext(nc) as tc, ExitStack() as ctx:
        eps_ap = eps_d.ap()

        rowsq = ctx.enter_context(tc.tile_pool(name="rowsq", bufs=4))
        work = ctx.enter_context(tc.tile_pool(name="work", bufs=5))
        workh = ctx.enter_context(tc.tile_pool(name="workh", bufs=2))
        cache = ctx.enter_context(tc.tile_pool(name="cache", bufs=1))
        stat = ctx.enter_context(tc.tile_pool(name="stat", bufs=1))
        smal = ctx.enter_context(tc.tile_pool(name="smal", bufs=1))
        psum = ctx.enter_context(tc.tile_pool(name="psum", bufs=1, space="PSUM"))

        # fp16 clipped-values cache: 64 tiles of [128, 1024] packed into
        # one persistent tile (128 KiB per partition)
        vcache = cache.tile([P, NBLK * NCHUNK * CHUNK], F16, tag="vc",
                            name="vcache")

        # ---------------- pass A: local scores ----------------
        # block-major so block 0's scores can AllGather while block 1
        # computes.  eps DMAs land directly in u_t (SP queue); c0/q
        # broadcasts ride the ACT/Pool DMA queues.
        sa_cols = [
            stat.tile([P, NCHUNK], F32, tag=f"sa{b}", name=f"sa_cols{b}")
            for b in range(NBLK)
        ]
        sb_cols = [
            stat.tile([P, NCHUNK], F32, tag=f"sb{b}", name=f"sb_cols{b}")
            for b in range(NBLK)
        ]
        s_loc = stat.tile([P, NBLK], F32, tag="sloc", name="s_loc")
        for b in range(NBLK):
            for k in range(NCHUNK):
                sl = slice(k * CHUNK, (k + 1) * CHUNK)
                q_t = rowsq.tile([P, CHUNK], F16, tag="q", name="q_t")
                nc.gpsimd.dma_start(
                    out=q_t[:], in_=qp_d.ap()[sl].partition_broadcast(P)
                )
                c0_t = rowsq.tile([P, CHUNK], F32, tag="c0", name="c0_t")
                nc.scalar.dma_start(
                    out=c0_t[:], in_=c0_d.ap()[sl].partition_broadcast(P)
                )
                u_t = work.tile([P, CHUNK], F32, tag="u", name="u_t")
                nc.sync.dma_start(out=u_t[:], in_=eps_ap[b * P:(b + 1) * P, sl])
                nc.vector.scalar_tensor_tensor(
                    out=u_t[:], in0=u_t[:], scalar=0.0, in1=c0_t[:],
                    op0=ALU.add, op1=ALU.add,
                )
                off = (k * NBLK + b) * CHUNK
                vsl = vcache[:, off:off + CHUNK]
                nc.vector.tensor_scalar(
                    out=vsl, in0=u_t[:], scalar1=inv_sig, scalar2=-inv_sig,
                    op0=ALU.min, op1=ALU.max,
                )
                d_t = workh.tile([P, CHUNK], F16, tag="d", name="d_t")
                nc.vector.scalar_tensor_tensor(
                    out=d_t[:], in0=vsl, scalar=1.0, in1=q_t[:],
                    op0=ALU.mult, op1=ALU.mult,
                    accum_out=sb_cols[b][:, k:k + 1],
                )
                d2 = workh.tile([P, CHUNK], F16, tag="d", name="d2")
                nc.scalar.activation(
                    out=d2[:], in_=vsl, func=ACTF.Square,
                    accum_out=sa_cols[b][:, k:k + 1],
                )
            # block-b scores -> DRAM -> AllGather (overlaps next block)
            sa_tot = smal.tile([P, 1], F32, tag="sat", name="sa_tot")
            nc.vector.tensor_reduce(sa_tot[:], sa_cols[b][:], axis=AX.X, op=ALU.add)
            sb_tot = smal.tile([P, 1], F32, tag="sbt", name="sb_tot")
            nc.vector.tensor_reduce(sb_tot[:], sb_cols[b][:], axis=AX.X, op=ALU.add)
            dtot = smal.tile([P, 1], F32, tag="dtot", name="dtot")
            nc.vector.scalar_tensor_tensor(
                out=dtot[:], in0=sb_tot[:], scalar=-2.0, in1=sa_tot[:],
                op0=ALU.mult, op1=ALU.add,
            )
            nc.vector.tensor_scalar_mul(s_loc[:, b:b + 1], dtot[:], cA)
            nc.sync.dma_start(
                out=sc_loc_d.ap()[b * P:(b + 1) * P]
                .rearrange("(a p) -> p a", a=1),
                in_=s_loc[:, b:b + 1],
            )
            if stage >= 2:
                nc.gpsimd.collective_compute(
                    "AllGather", ALU.bypass,
                    ins=[sc_loc_d.ap()[b * P:(b + 1) * P]],
                    outs=[sc_all_d.ap()[b * P * N_CORES:(b + 1) * P * N_CORES]],
                    replica_groups=rg,
                )
        if stage <= 1:
            nc.sync.dma_start(
                out=out_d.ap()[0:NLOC].rearrange("(b p) -> p b", p=P),
                in_=s_loc[:],
            )

        # ---------------- softmax stats ----------------
        # weights are UN-normalized exp() here; the global Z rides the
        # AllReduce (slot TF of ws) and division happens post-reduce.
        wt16 = None
        if stage >= 2:
            onesr = smal.tile([1, P], F32, tag="onesr", name="onesr")
            nc.sync.dma_start(
                out=onesr[:], in_=ones_d.ap().rearrange("(a n) -> a n", a=1)
            )
            onec = smal.tile([P, 1], F32, tag="onec", name="onec")
            nc.sync.dma_start(
                out=onec[:], in_=ones_d.ap().rearrange("(p a) -> p a", a=1)
            )
            s_all = smal.tile([1, N], F32, tag="sall", name="s_all")
            nc.sync.dma_start(
                out=s_all[:], in_=sc_all_d.ap().rearrange("(a n) -> a n", a=1)
            )
            pack = smal.tile([1, 2], F32, tag="pack", name="pack")
            negmean = smal.tile([1, 1], F32, tag="negmean", name="negmean")
            nc.vector.tensor_reduce(negmean[:], s_all[:], axis=AX.X, op=ALU.add)
            nc.vector.tensor_scalar_mul(negmean[:], negmean[:], -1.0 / N)
            js = smal.tile([1, N], F16, tag="js", name="js")
            ssq = smal.tile([1, 1], F32, tag="ssq", name="ssq")
            nc.scalar.activation(
                out=js[:], in_=s_all[:], func=ACTF.Square, bias=negmean[:],
                accum_out=ssq[:],
            )
            # std = max(sqrt(ssq/(N-1)), 1e-4); pack0 = 1/(std*TEMP)
            std = smal.tile([1, 1], F32, tag="std", name="std")
            nc.scalar.activation(
                out=std[:], in_=ssq[:], func=ACTF.Sqrt, scale=1.0 / (N - 1)
            )
            stdT = smal.tile([1, 1], F32, tag="stdT", name="stdT")
            nc.vector.tensor_scalar(
                out=stdT[:], in0=std[:], scalar1=1e-4, scalar2=TEMP,
                op0=ALU.max, op1=ALU.mult,
            )
            nc.vector.reciprocal(pack[:, 0:1], stdT[:])
            mx = smal.tile([1, 1], F32, tag="mx", name="mx")
            nc.vector.tensor_reduce(mx[:], s_all[:], axis=AX.X, op=ALU.max)
            # shifted logit: (s - mx)*inv10 (mean cancels in the shift, and
            # the un-normalized exp is safe: max exponent is exactly 0)
            nmx = smal.tile([1, 1], F32, tag="nmx", name="nmx")
            nc.vector.tensor_scalar_mul(nmx[:], mx[:], -1.0)
            nc.vector.tensor_tensor(pack[:, 1:2], nmx[:], pack[:, 0:1], ALU.mult)
            # PE-broadcast (inv10, bg) to all 128 partitions
            bps = psum.tile([P, 2], F32, tag="bps", bufs=1, name="bps")
            nc.tensor.matmul(bps[:], lhsT=onesr[:], rhs=pack[:], start=True,
                             stop=True)
            scal = smal.tile([P, 2], F32, tag="scal", name="scal")
            nc.vector.tensor_copy(scal[:], bps[:])

            # warm the PE p-state before pass B: tiny back-to-back dummy
            # matmuls gated on the post-stats scal tile keep the PE busy
            # ~3us so the real fp16 matmuls run at 2.4 GHz, not 1.2
            jl = smal.tile([P, 1], F16, tag="jl", name="jl")
            nc.scalar.copy(jl[:], scal[:, 0:1])
            for w in range(35):
                wmm = psum.tile([P, 2], F32, tag="bps", bufs=1, name="wmm")
                nc.tensor.matmul(wmm[0:1, 0:1], lhsT=jl[:], rhs=jl[:],
                                 start=True, stop=True)

            e_loc = smal.tile([P, NBLK], F32, tag="eloc", name="e_loc")
            nc.scalar.activation(
                out=e_loc[:], in_=s_loc[:], func=ACTF.Exp,
                scale=scal[:, 0:1], bias=scal[:, 1:2],
            )
            wt16 = stat.tile([P, NBLK], F16, tag="wt16", name="wt16")
            zloc = smal.tile([P, 1], F32, tag="zloc", name="zloc")
            nc.scalar.activation(
                out=wt16[:], in_=e_loc[:], func=ACTF.Copy, accum_out=zloc[:]
            )
            # local Z -> ws_loc[TF] so the AllReduce sums it globally
            zpt = psum.tile([P, 1], F32, tag="qps", bufs=1, name="zpt")
            zps = zpt[0:1, 0:1]
            nc.tensor.matmul(zps, lhsT=zloc[:], rhs=onec[:], start=True,
                             stop=True)
            ztot = smal.tile([1, 1], F32, tag="ztot", name="ztot")
            nc.vector.tensor_copy(ztot[:], zps)
            nc.sync.dma_start(
                out=ws_loc_d.ap()[TF:TF + 1].rearrange("(a n) -> a n", a=1),
                in_=ztot[:],
            )
            if stage <= 2:
                nc.sync.dma_start(
                    out=out_d.ap()[0:NLOC].rearrange("(b p) -> p b", p=P),
                    in_=e_loc[:],
                )

        # ---------------- pass B: weighted sum on PE from SBUF cache ----
        if stage >= 3:
            # two 512-wide PSUM rows (= one 1024 chunk) per bounce tile:
            # halves the copy and writeback-DMA count
            for k in range(NCHUNK):
                wrow = psum.tile([1, CHUNK], F32, tag="wrow", bufs=3,
                                 name="wrow")
                for half in range(2):
                    for b in range(NBLK):
                        off = (k * NBLK + b) * CHUNK + half * SUB
                        nc.tensor.matmul(
                            wrow[:, half * SUB:(half + 1) * SUB],
                            lhsT=wt16[:, b:b + 1],
                            rhs=vcache[:, off:off + SUB],
                            start=(b == 0), stop=(b == NBLK - 1),
                        )
                wsb = work.tile([1, CHUNK], F32, tag="wsb", bufs=3, name="wsb")
                if k % 2 == 0:
                    nc.vector.tensor_copy(wsb[:], wrow[:])
                else:
                    nc.scalar.copy(wsb[:], wrow[:])
                nc.sync.dma_start(
                    out=ws_loc_d.ap()[k * CHUNK:(k + 1) * CHUNK]
                    .rearrange("(a n) -> a n", a=1),
                    in_=wsb[:],
                )
            if stage <= 3:
                o3 = stat.tile([P, TF // P], F32, tag="o3", name="o3")
                nc.sync.dma_start(
                    out=o3[:],
                    in_=ws_loc_d.ap()[0:TF].rearrange("(p c) -> p c", p=P),
                )
                nc.sync.dma_start(
                    out=out_d.ap().rearrange("(p c) -> p c", p=P), in_=o3[:]
                )

        # ---------------- AllReduce + final combine ----------------
        if stage >= 4:
            # obs/mask preloads don't depend on anything — issue early is
            # handled by the scheduler; they're plain loads.
            rowmaj0 = lambda d: d.ap()[0:TF].rearrange("(p c) -> p c", p=P)
            obs_t = stat.tile([P, TF // P], F32, tag="obsf", name="obs_t")
            nc.sync.dma_start(out=obs_t[:], in_=rowmaj0(obs_d))
            m_t = stat.tile([P, TF // P], F32, tag="mf", name="m_t")
            nc.sync.dma_start(out=m_t[:], in_=rowmaj0(maskf_d))
            nc.gpsimd.collective_compute(
                "AllReduce", ALU.add,
                ins=[ws_loc_d.ap()], outs=[ws_all_d.ap()], replica_groups=rg,
            )
            w_t = stat.tile([P, TF // P], F32, tag="wfin", name="w_t")
            nc.sync.dma_start(out=w_t[:], in_=rowmaj0(ws_all_d))
            zg = smal.tile([1, 1], F32, tag="zg", name="zg")
            nc.sync.dma_start(
                out=zg[:],
                in_=ws_all_d.ap()[TF:TF + 1].rearrange("(a n) -> a n", a=1),
            )
            rzg = smal.tile([1, 1], F32, tag="rzg", name="rzg")
            nc.vector.reciprocal(rzg[:], zg[:])
            qfin = smal.tile([1, 1], F32, tag="qfin", name="qfin")
            nc.vector.tensor_scalar_mul(qfin[:], rzg[:], float(c1 * sigma_i))
            qps = psum.tile([P, 1], F32, tag="qps", bufs=1, name="qps")
            nc.tensor.matmul(qps[:], lhsT=onesr[:], rhs=qfin[:], start=True,
                             stop=True)
            qb = smal.tile([P, 1], F32, tag="qb", name="qb")
            nc.vector.tensor_copy(qb[:], qps[:])
            t1 = stat.tile([P, TF // P], F32, tag="t1", name="t1")
            nc.vector.tensor_single_scalar(
                out=t1[:], in_=w_t[:], scalar=qb[:], op=ALU.mult
            )
            # in-place: obs_t <- m*(obs - t1); t1 <- t1 + obs_t
            nc.vector.tensor_tensor(obs_t[:], obs_t[:], t1[:], ALU.subtract)
            nc.vector.tensor_tensor(obs_t[:], obs_t[:], m_t[:], ALU.mult)
            nc.vector.tensor_tensor(t1[:], t1[:], obs_t[:], ALU.add)
            nc.sync.dma_start(out=rowmaj0(out_d), in_=t1[:])

    nc.compile()
    return nc


_CACHE: dict = {}
TRACE = False
STAGE = 4
LAST_RESULTS = None


def kernel(Xbar_i, observed_data, time_points, mask, eps, deg_a, deg_b, i):
    global LAST_RESULTS
    i = int(i)
    sigma_i, c1 = _schedule_scalars(i)
    key = ("v5", i, STAGE)
    if key not in _CACHE:
        _CACHE[key] = _build(float(sigma_i), float(c1), stage=STAGE)
    nc = _CACHE[key]

    inv_sig = np.float32(1.0) / sigma_i
    Xb = np.asarray(Xbar_i, np.float32)
    obs = np.asarray(observed_data, np.float32)
    msk = np.asarray(mask, bool)
    tp = np.asarray(time_points, np.float32)
    da = np.asarray(deg_a, np.float32)
    db = np.asarray(deg_b, np.float32)
    epsf = np.asarray(eps, np.float32)

    pred = da[None, :] + db[None, :] * tp[:, None]
    c0 = (Xb * inv_sig).astype(np.float32)
    c0 = np.where(msk, np.float32(1e6), c0).reshape(-1)
    qp = (pred * inv_sig).astype(np.float32)
    qp = np.where(msk, inv_sig, qp).reshape(-1).astype(np.float16)
    obsf = obs.reshape(-1)
    maskf = msk.astype(np.float32).reshape(-1)

    in_maps = []
    for c in range(N_CORES):
        shard = np.ascontiguousarray(
            epsf[c * NLOC:(c + 1) * NLOC].reshape(NLOC, TF)
        )
        in_maps.append(
            {"eps": shard, "c0": c0, "qp": qp, "obs": obsf, "maskf": maskf,
             "ones": np.ones(128, np.float32)}
        )
    kr = run_bass_kernel_spmd(nc, in_maps, list(range(N_CORES)), trace=TRACE)
    LAST_RESULTS = kr
    return kr.results[0]["out"].reshape(T, F).astype(np.float32)



# revision 9
# speedup vs baseline: 1.3076x; 1.3076x over previous
"""MBD degradation-imputation sampling step on 8 Trainium2 NeuronCores.

v6 strategy (data-parallel over N=2048 candidates, 256/core), sample-major
tiles [128 samples, 1024 tf], chunk-major loop:

  pass A : ONE HBM pass over eps (~34 MB/core, the DMA roofline).  The
           c0/q' per-chunk rows are broadcast to 128 partitions by the
           TensorEngine (K=1 matmuls from a preloaded [32,2048] fp16
           tile) instead of 48 MB of partition_broadcast DMA (the v5
           bottleneck: 85 MB SBUF writes, DMA 72% busy).
           Engine split per [128,1024] tile:
             DVE    : u16 = eps + c0  (TT fp32+PSUM -> fp16, 1.04 ns/e)
                      v16 = clip(u16) (TS fp16 4x mode, 0.26 ns/e) -> vcache
             ACT    : Sa += sum(v16^2)     (Square + accum_out)
             GpSimd : Sb += sum(v16*q16)   (STT + accum_out, SBUF-only)
           score = cA*(Sa - 2*Sb) + sample-independent shift; observed
           positions saturate (c0=6e4 -> v=m, q=m) and cancel.
  softmax: single 1 KB AllGather of all 256 local scores, stats on the
           gathered 2048, un-normalized exp(); local Z rides slot TF of
           the AllReduce buffer.
  pass B : weighted partition-reduction from the fp16 vcache on the
           TensorEngine (M=1 fp16 matmuls, PSUM-accumulated), bounce
           split Vector/Scalar, AllReduce (T,F)+Z, final combine.

PSUM: one [128,1024] f32 tag rotating 3 buffers serves the c0/q
broadcasts in pass A and wrow/bps/qps (as slices) afterwards.

`stage` truncates for bisection: 1 = pass A only, 2 = +AG/softmax,
3 = +pass B (no AllReduce), 4 = full kernel.
"""

from contextlib import ExitStack

import numpy as np

import concourse.bass as bass
import concourse.tile as tile
from concourse import bacc, mybir
from concourse.bass_utils import run_bass_kernel_spmd

N_CORES = 8
N, T, F = 2048, 512, 64
P = 128
TF = T * F                      # 32768
NLOC = N // N_CORES             # 256
NBLK = NLOC // P                # 2
CHUNK = 1024
NCHUNK = TF // CHUNK            # 32
SUB = 512                       # matmul N (one PSUM bank)
TEMP = 0.1
T_STEPS = 1000
SAT = 60000.0                   # fp16-exact saturation for observed c0

F32 = mybir.dt.float32
F16 = mybir.dt.float16
AX = mybir.AxisListType
ALU = mybir.AluOpType
ACTF = mybir.ActivationFunctionType

# chunks whose d=v-q subtract runs on DVE instead of GpSimd (Pool only
# supports plain TensorTensor; tune the split so neither engine exceeds
# the ~101us eps-DMA window: DVE has ~15us slack, GpSimd does the rest)
SUB_DVE_MOD = 3                 # k % MOD == 0 -> subtract on DVE


def _schedule_scalars(i: int):
    s = 0.008
    x = np.linspace(0, T_STEPS, T_STEPS + 1, dtype=np.float64)
    ac = np.cos((x / T_STEPS + s) / (1 + s) * np.pi * 0.5) ** 2
    ac = ac / ac[0]
    betas = np.clip(1.0 - ac[1:] / ac[:-1], 0.0, 0.999)
    alphas = 1.0 - betas
    acp = np.cumprod(alphas)
    abar_i = np.float32(acp[i])
    sigma_i = np.float32(np.sqrt(1.0 - acp[i]))
    alpha_i = np.float32(alphas[i])
    abar_im1 = np.float32(acp[i - 1])
    sa = np.float32(np.sqrt(abar_i))
    # the reference's Yi terms cancel exactly; out_missing = c1 * weighted
    c1 = np.float32(sa / np.float32(np.sqrt(alpha_i)) / np.float32(np.sqrt(abar_im1)))
    return sigma_i, c1


def _build(sigma_i: float, c1: float, stage: int = 4):
    inv_sig = float(np.float32(1.0 / np.float32(sigma_i)))
    sigma_i = float(np.float32(sigma_i))
    c1 = float(np.float32(c1))
    # scores = cA * sum((v - q')^2)  (+ sample-independent shift vs ref)
    cA = float(np.float32(-(np.float32(sigma_i) ** 2) / np.float32(TF)))

    nc = bacc.Bacc(
        "TRN2", target_bir_lowering=False, debug=False, num_devices=N_CORES
    )
    eps_d = nc.dram_tensor("eps", [NLOC, TF], F32, kind="ExternalInput")
    # cq16[k] = [c0 chunk k (1024) | q chunk k (1024)] fp16
    cq_d = nc.dram_tensor("cq16", [NCHUNK, 2 * CHUNK], F16, kind="ExternalInput")
    obs_d = nc.dram_tensor("obs", [TF], F32, kind="ExternalInput")
    maskf_d = nc.dram_tensor("maskf", [TF], F32, kind="ExternalInput")
    out_d = nc.dram_tensor("out", [TF], F32, kind="ExternalOutput")

    ones_d = nc.dram_tensor("ones", [P], F32, kind="ExternalInput")
    ones16_d = nc.dram_tensor("ones16", [P], F16, kind="ExternalInput")
    sc_loc_d = nc.dram_tensor("sc_loc", [NLOC], F32)
    sc_all_d = nc.dram_tensor("sc_all", [N], F32, addr_space="Shared")
    # ws carries the TF weighted partials plus the local softmax
    # normalizer Z in slot TF — one AllReduce delivers both.
    ws_loc_d = nc.dram_tensor("ws_loc", [TF + 4], F32)
    ws_all_d = nc.dram_tensor("ws_all", [TF + 4], F32, addr_space="Shared")

    rg = [list(range(N_CORES))]

    with tile.TileContext(nc) as tc, ExitStack() as ctx:
        eps_ap = eps_d.ap()

        rowsq = ctx.enter_context(tc.tile_pool(name="rowsq", bufs=1))
        work = ctx.enter_context(tc.tile_pool(name="work", bufs=3))
        cache = ctx.enter_context(tc.tile_pool(name="cache", bufs=1))
        stat = ctx.enter_context(tc.tile_pool(name="stat", bufs=1))
        smal = ctx.enter_context(tc.tile_pool(name="smal", bufs=1))
        psum = ctx.enter_context(tc.tile_pool(name="psum", bufs=3, space="PSUM"))

        def pstile(name):
            return psum.tile([P, CHUNK], F32, tag="ps", bufs=3, name=name)

        # fp16 clipped-values cache: 64 tiles of [128, 1024] packed into
        # one persistent tile (128 KiB per partition)
        vcache = cache.tile([P, NBLK * NCHUNK * CHUNK], F16, tag="vc",
                            name="vcache")

        # fp16 ones row for the K=1 broadcast matmuls
        onesr16 = smal.tile([1, P], F16, tag="onesr16", name="onesr16")
        nc.sync.dma_start(
            out=onesr16[:], in_=ones16_d.ap().rearrange("(a n) -> a n", a=1)
        )
        # per-chunk [c0|q] row goes to partition 0 (matmul rhs needs base
        # partition 0); one 4 KB DMA per chunk

        # ---------------- pass A: local scores ----------------
        sd_cols = [
            stat.tile([P, NCHUNK], F32, tag=f"sd{b}", name=f"sd_cols{b}")
            for b in range(NBLK)
        ]
        s_loc = stat.tile([P, NBLK], F32, tag="sloc", name="s_loc")
        for k in range(NCHUNK):
            cqr = rowsq.tile([1, 2 * CHUNK], F16, tag="cqr", bufs=3,
                             name="cqr")
            nc.sync.dma_start(out=cqr[:], in_=cq_d.ap()[k:k + 1, :])
            # PE broadcast of c0/q rows (K=1 matmuls, fp16 in, f32 PSUM;
            # one matmul per 512-wide PSUM bank)
            psC = pstile("psC")
            psQ = pstile("psQ")
            for h in range(2):
                hs = slice(h * SUB, (h + 1) * SUB)
                nc.tensor.matmul(psC[:, hs], lhsT=onesr16[:],
                                 rhs=cqr[0:1, h * SUB:(h + 1) * SUB],
                                 start=True, stop=True)
                nc.tensor.matmul(psQ[:, hs], lhsT=onesr16[:],
                                 rhs=cqr[0:1, CHUNK + h * SUB:
                                             CHUNK + (h + 1) * SUB],
                                 start=True, stop=True)
            q16b = work.tile([P, CHUNK], F16, tag="q16b", bufs=2, name="q16b")
            nc.scalar.activation(out=q16b[:], in_=psQ[:], func=ACTF.Copy)
            sl = slice(k * CHUNK, (k + 1) * CHUNK)
            for b in range(NBLK):
                u_t = work.tile([P, CHUNK], F32, tag="u", bufs=3, name="u_t")
                nc.sync.dma_start(out=u_t[:], in_=eps_ap[b * P:(b + 1) * P, sl])
                u16 = work.tile([P, CHUNK], F16, tag="u16", bufs=2, name="u16")
                nc.vector.tensor_tensor(u16[:], u_t[:], psC[:], ALU.add)
                off = (k * NBLK + b) * CHUNK
                vsl = vcache[:, off:off + CHUNK]
                nc.vector.tensor_scalar(
                    out=vsl, in0=u16[:], scalar1=inv_sig, scalar2=-inv_sig,
                    op0=ALU.min, op1=ALU.max,
                )
                d16 = work.tile([P, CHUNK], F16, tag="d16", bufs=3,
                                name="d16")
                eng = (nc.vector if (SUB_DVE_MOD and k % SUB_DVE_MOD == 0)
                       else nc.gpsimd)
                eng.tensor_tensor(d16[:], vsl, q16b[:], ALU.subtract)
                d2 = work.tile([P, CHUNK], F16, tag="d2", bufs=2, name="d2")
                nc.scalar.activation(
                    out=d2[:], in_=d16[:], func=ACTF.Square,
                    accum_out=sd_cols[b][:, k:k + 1],
                )
        # score finalize: s = cA*sum(d^2)
        for b in range(NBLK):
            sd_tot = smal.tile([P, 1], F32, tag="sdt", name="sd_tot")
            nc.vector.tensor_reduce(sd_tot[:], sd_cols[b][:], axis=AX.X, op=ALU.add)
            nc.vector.tensor_scalar_mul(s_loc[:, b:b + 1], sd_tot[:], cA)
        nc.sync.dma_start(
            out=sc_loc_d.ap().rearrange("(b p) -> p b", b=NBLK),
            in_=s_loc[:],
        )
        if stage >= 2:
            nc.gpsimd.collective_compute(
                "AllGather", ALU.bypass,
                ins=[sc_loc_d.ap()],
                outs=[sc_all_d.ap()],
                replica_groups=rg,
            )
        if stage <= 1:
            nc.sync.dma_start(
                out=out_d.ap()[0:NLOC].rearrange("(b p) -> p b", p=P),
                in_=s_loc[:],
            )

        # ---------------- softmax stats ----------------
        # weights are UN-normalized exp(); the global Z rides the
        # AllReduce (slot TF of ws) and division happens post-reduce.
        wt16 = None
        if stage >= 2:
            onesr = smal.tile([1, P], F32, tag="onesr", name="onesr")
            nc.sync.dma_start(
                out=onesr[:], in_=ones_d.ap().rearrange("(a n) -> a n", a=1)
            )
            onec = smal.tile([P, 1], F32, tag="onec", name="onec")
            nc.sync.dma_start(
                out=onec[:], in_=ones_d.ap().rearrange("(p a) -> p a", a=1)
            )
            s_all = smal.tile([1, N], F32, tag="sall", name="s_all")
            nc.sync.dma_start(
                out=s_all[:], in_=sc_all_d.ap().rearrange("(a n) -> a n", a=1)
            )
            pack = smal.tile([1, 2], F32, tag="pack", name="pack")
            negmean = smal.tile([1, 1], F32, tag="negmean", name="negmean")
            nc.vector.tensor_reduce(negmean[:], s_all[:], axis=AX.X, op=ALU.add)
            nc.vector.tensor_scalar_mul(negmean[:], negmean[:], -1.0 / N)
            js = smal.tile([1, N], F16, tag="js", name="js")
            ssq = smal.tile([1, 1], F32, tag="ssq", name="ssq")
            nc.scalar.activation(
                out=js[:], in_=s_all[:], func=ACTF.Square, bias=negmean[:],
                accum_out=ssq[:],
            )
            # std = max(sqrt(ssq/(N-1)), 1e-4); pack0 = 1/(std*TEMP)
            std = smal.tile([1, 1], F32, tag="std", name="std")
            nc.scalar.activation(
                out=std[:], in_=ssq[:], func=ACTF.Sqrt, scale=1.0 / (N - 1)
            )
            stdT = smal.tile([1, 1], F32, tag="stdT", name="stdT")
            nc.vector.tensor_scalar(
                out=stdT[:], in0=std[:], scalar1=1e-4, scalar2=TEMP,
                op0=ALU.max, op1=ALU.mult,
            )
            nc.vector.reciprocal(pack[:, 0:1], stdT[:])
            mx = smal.tile([1, 1], F32, tag="mx", name="mx")
            nc.vector.tensor_reduce(mx[:], s_all[:], axis=AX.X, op=ALU.max)
            # shifted logit: (s - mx)*inv10 (mean cancels in the shift, and
            # the un-normalized exp is safe: max exponent is exactly 0)
            nmx = smal.tile([1, 1], F32, tag="nmx", name="nmx")
            nc.vector.tensor_scalar_mul(nmx[:], mx[:], -1.0)
            nc.vector.tensor_tensor(pack[:, 1:2], nmx[:], pack[:, 0:1], ALU.mult)
            # PE-broadcast (inv10, bg) to all 128 partitions
            bps = pstile("bps")
            nc.tensor.matmul(bps[:, 0:2], lhsT=onesr[:], rhs=pack[:],
                             start=True, stop=True)
            scal = smal.tile([P, 2], F32, tag="scal", name="scal")
            nc.vector.tensor_copy(scal[:], bps[:, 0:2])

            # warm the PE p-state before pass B: tiny back-to-back dummy
            # matmuls gated on the post-stats scal tile keep the PE busy
            # ~3us so the real fp16 matmuls run at 2.4 GHz, not 1.2
            jl = smal.tile([P, 1], F16, tag="jl", name="jl")
            nc.scalar.copy(jl[:], scal[:, 0:1])
            for w in range(35):
                wmm = pstile("wmm")
                nc.tensor.matmul(wmm[0:1, 0:1], lhsT=jl[:], rhs=jl[:],
                                 start=True, stop=True)

            e_loc = smal.tile([P, NBLK], F32, tag="eloc", name="e_loc")
            nc.scalar.activation(
                out=e_loc[:], in_=s_loc[:], func=ACTF.Exp,
                scale=scal[:, 0:1], bias=scal[:, 1:2],
            )
            wt16 = stat.tile([P, NBLK], F16, tag="wt16", name="wt16")
            zloc = smal.tile([P, 1], F32, tag="zloc", name="zloc")
            nc.scalar.activation(
                out=wt16[:], in_=e_loc[:], func=ACTF.Copy, accum_out=zloc[:]
            )
            # local Z -> ws_loc[TF] so the AllReduce sums it globally
            zpt = pstile("zpt")
            zps = zpt[0:1, 0:1]
            nc.tensor.matmul(zps, lhsT=zloc[:], rhs=onec[:], start=True,
                             stop=True)
            ztot = smal.tile([1, 1], F32, tag="ztot", name="ztot")
            nc.vector.tensor_copy(ztot[:], zps)
            nc.sync.dma_start(
                out=ws_loc_d.ap()[TF:TF + 1].rearrange("(a n) -> a n", a=1),
                in_=ztot[:],
            )
            if stage <= 2:
                nc.sync.dma_start(
                    out=out_d.ap()[0:NLOC].rearrange("(b p) -> p b", p=P),
                    in_=e_loc[:],
                )

        # ---------------- pass B: weighted sum on PE from SBUF cache ----
        if stage >= 3:
            # two 512-wide PSUM rows (= one 1024 chunk) per bounce tile:
            # halves the copy and writeback-DMA count
            for k in range(NCHUNK):
                wrow = pstile("wrow")
                for half in range(2):
                    for b in range(NBLK):
                        off = (k * NBLK + b) * CHUNK + half * SUB
                        nc.tensor.matmul(
                            wrow[0:1, half * SUB:(half + 1) * SUB],
                            lhsT=wt16[:, b:b + 1],
                            rhs=vcache[:, off:off + SUB],
                            start=(b == 0), stop=(b == NBLK - 1),
                        )
                wsb = work.tile([1, CHUNK], F32, tag="wsb", bufs=3, name="wsb")
                if k % 2 == 0:
                    nc.vector.tensor_copy(wsb[:], wrow[0:1, :])
                else:
                    nc.scalar.copy(wsb[:], wrow[0:1, :])
                nc.sync.dma_start(
                    out=ws_loc_d.ap()[k * CHUNK:(k + 1) * CHUNK]
                    .rearrange("(a n) -> a n", a=1),
                    in_=wsb[:],
                )
            if stage <= 3:
                o3 = stat.tile([P, TF // P], F32, tag="o3", name="o3")
                nc.sync.dma_start(
                    out=o3[:],
                    in_=ws_loc_d.ap()[0:TF].rearrange("(p c) -> p c", p=P),
                )
                nc.sync.dma_start(
                    out=out_d.ap().rearrange("(p c) -> p c", p=P), in_=o3[:]
                )

        # ---------------- AllReduce + final combine ----------------
        if stage >= 4:
            rowmaj0 = lambda d: d.ap()[0:TF].rearrange("(p c) -> p c", p=P)
            obs_t = stat.tile([P, TF // P], F32, tag="obsf", name="obs_t")
            nc.sync.dma_start(out=obs_t[:], in_=rowmaj0(obs_d))
            m_t = stat.tile([P, TF // P], F32, tag="mf", name="m_t")
            nc.sync.dma_start(out=m_t[:], in_=rowmaj0(maskf_d))
            nc.gpsimd.collective_compute(
                "AllReduce", ALU.add,
                ins=[ws_loc_d.ap()], outs=[ws_all_d.ap()], replica_groups=rg,
            )
            w_t = stat.tile([P, TF // P], F32, tag="wfin", name="w_t")
            nc.sync.dma_start(out=w_t[:], in_=rowmaj0(ws_all_d))
            zg = smal.tile([1, 1], F32, tag="zg", name="zg")
            nc.sync.dma_start(
                out=zg[:],
                in_=ws_all_d.ap()[TF:TF + 1].rearrange("(a n) -> a n", a=1),
            )
            rzg = smal.tile([1, 1], F32, tag="rzg", name="rzg")
            nc.vector.reciprocal(rzg[:], zg[:])
            qfin = smal.tile([1, 1], F32, tag="qfin", name="qfin")
            nc.vector.tensor_scalar_mul(qfin[:], rzg[:], float(c1 * sigma_i))
            qps = pstile("qps")
            nc.tensor.matmul(qps[:, 0:1], lhsT=onesr[:], rhs=qfin[:],
                             start=True, stop=True)
            qb = smal.tile([P, 1], F32, tag="qb", name="qb")
            nc.vector.tensor_copy(qb[:], qps[:, 0:1])
            t1 = stat.tile([P, TF // P], F32, tag="t1", name="t1")
            nc.vector.tensor_single_scalar(
                out=t1[:], in_=w_t[:], scalar=qb[:], op=ALU.mult
            )
            # in-place: obs_t <- m*(obs - t1); t1 <- t1 + obs_t
            nc.vector.tensor_tensor(obs_t[:], obs_t[:], t1[:], ALU.subtract)
            nc.vector.tensor_tensor(obs_t[:], obs_t[:], m_t[:], ALU.mult)
            nc.vector.tensor_tensor(t1[:], t1[:], obs_t[:], ALU.add)
            nc.sync.dma_start(out=rowmaj0(out_d), in_=t1[:])

    nc.compile()
    return nc


_CACHE: dict = {}
TRACE = False
STAGE = 4
LAST_RESULTS = None


def kernel(Xbar_i, observed_data, time_points, mask, eps, deg_a, deg_b, i):
    global LAST_RESULTS
    i = int(i)
    sigma_i, c1 = _schedule_scalars(i)
    key = ("v6", i, STAGE, SUB_DVE_MOD)
    if key not in _CACHE:
        _CACHE[key] = _build(float(sigma_i), float(c1), stage=STAGE)
    nc = _CACHE[key]

    inv_sig = np.float32(1.0) / sigma_i
    Xb = np.asarray(Xbar_i, np.float32)
    obs = np.asarray(observed_data, np.float32)
    msk = np.asarray(mask, bool)
    tp = np.asarray(time_points, np.float32)
    da = np.asarray(deg_a, np.float32)
    db = np.asarray(deg_b, np.float32)
    epsf = np.asarray(eps, np.float32)

    pred = da[None, :] + db[None, :] * tp[:, None]
    c0 = (Xb * inv_sig).astype(np.float32)
    c0 = np.where(msk, np.float32(SAT), c0).reshape(-1)
    qp = (pred * inv_sig).astype(np.float32)
    qp = np.where(msk, inv_sig, qp).reshape(-1)
    # cq16[k] = [c0 chunk | q chunk] fp16
    cq16 = np.concatenate(
        [c0.reshape(NCHUNK, CHUNK), qp.reshape(NCHUNK, CHUNK)], axis=1
    ).astype(np.float16)
    obsf = obs.reshape(-1)
    maskf = msk.astype(np.float32).reshape(-1)

    in_maps = []
    for c in range(N_CORES):
        shard = np.ascontiguousarray(
            epsf[c * NLOC:(c + 1) * NLOC].reshape(NLOC, TF)
        )
        in_maps.append(
            {"eps": shard, "cq16": cq16, "obs": obsf, "maskf": maskf,
             "ones": np.ones(P, np.float32),
             "ones16": np.ones(P, np.float16)}
        )
    kr = run_bass_kernel_spmd(nc, in_maps, list(range(N_CORES)), trace=TRACE)
    LAST_RESULTS = kr
    return kr.results[0]["out"].reshape(T, F).astype(np.float32)


# revision 12
# speedup vs baseline: 1.3614x; 1.0411x over previous
"""MBD degradation-imputation sampling step on 8 Trainium2 NeuronCores.

v6 strategy (data-parallel over N=2048 candidates, 256/core), sample-major
tiles [128 samples, 1024 tf], chunk-major loop:

  pass A : ONE HBM pass over eps (~34 MB/core, the DMA roofline).  The
           c0/q' per-chunk rows are broadcast to 128 partitions by the
           TensorEngine (K=1 matmuls from a preloaded [32,2048] fp16
           tile) instead of 48 MB of partition_broadcast DMA (the v5
           bottleneck: 85 MB SBUF writes, DMA 72% busy).
           Engine split per [128,1024] tile:
             DVE    : u16 = eps + c0  (TT fp32+PSUM -> fp16, 1.04 ns/e)
                      v16 = clip(u16) (TS fp16 4x mode, 0.26 ns/e) -> vcache
             ACT    : Sa += sum(v16^2)     (Square + accum_out)
             GpSimd : Sb += sum(v16*q16)   (STT + accum_out, SBUF-only)
           score = cA*(Sa - 2*Sb) + sample-independent shift; observed
           positions saturate (c0=6e4 -> v=m, q=m) and cancel.
  softmax: single 1 KB AllGather of all 256 local scores, stats on the
           gathered 2048, un-normalized exp(); local Z rides slot TF of
           the AllReduce buffer.
  pass B : weighted partition-reduction from the fp16 vcache on the
           TensorEngine (M=1 fp16 matmuls, PSUM-accumulated), bounce
           split Vector/Scalar, AllReduce (T,F)+Z, final combine.

PSUM: one [128,1024] f32 tag rotating 3 buffers serves the c0/q
broadcasts in pass A and wrow/bps/qps (as slices) afterwards.

`stage` truncates for bisection: 1 = pass A only, 2 = +AG/softmax,
3 = +pass B (no AllReduce), 4 = full kernel.
"""

from contextlib import ExitStack

import numpy as np

import concourse.bass as bass
import concourse.tile as tile
from concourse import bacc, mybir
from concourse.bass_utils import run_bass_kernel_spmd

N_CORES = 8
N, T, F = 2048, 512, 64
P = 128
TF = T * F                      # 32768
NLOC = N // N_CORES             # 256
NBLK = NLOC // P                # 2
CHUNK = 1024
NCHUNK = TF // CHUNK            # 32
SUB = 512                       # matmul N (one PSUM bank)
TEMP = 0.1
T_STEPS = 1000
SAT = 60000.0                   # fp16-exact saturation for observed c0

F32 = mybir.dt.float32
F16 = mybir.dt.float16
AX = mybir.AxisListType
ALU = mybir.AluOpType
ACTF = mybir.ActivationFunctionType

# chunks whose d=v-q subtract runs on DVE instead of GpSimd (Pool only
# supports plain TensorTensor; tune the split so neither engine exceeds
# the ~101us eps-DMA window).  DVE subs read q straight from PSUM via
# STT (no SBUF q16 copy); GpSimd chunks need the ACT copy of q16.
SUB_DVE_MOD = 4                 # k % MOD == 0 -> subtract on DVE


def _schedule_scalars(i: int):
    s = 0.008
    x = np.linspace(0, T_STEPS, T_STEPS + 1, dtype=np.float64)
    ac = np.cos((x / T_STEPS + s) / (1 + s) * np.pi * 0.5) ** 2
    ac = ac / ac[0]
    betas = np.clip(1.0 - ac[1:] / ac[:-1], 0.0, 0.999)
    alphas = 1.0 - betas
    acp = np.cumprod(alphas)
    abar_i = np.float32(acp[i])
    sigma_i = np.float32(np.sqrt(1.0 - acp[i]))
    alpha_i = np.float32(alphas[i])
    abar_im1 = np.float32(acp[i - 1])
    sa = np.float32(np.sqrt(abar_i))
    # the reference's Yi terms cancel exactly; out_missing = c1 * weighted
    c1 = np.float32(sa / np.float32(np.sqrt(alpha_i)) / np.float32(np.sqrt(abar_im1)))
    return sigma_i, c1


def _build(sigma_i: float, c1: float, stage: int = 4):
    inv_sig = float(np.float32(1.0 / np.float32(sigma_i)))
    sigma_i = float(np.float32(sigma_i))
    c1 = float(np.float32(c1))
    # scores = cA * sum((v - q')^2)  (+ sample-independent shift vs ref)
    cA = float(np.float32(-(np.float32(sigma_i) ** 2) / np.float32(TF)))

    nc = bacc.Bacc(
        "TRN2", target_bir_lowering=False, debug=False, num_devices=N_CORES
    )
    eps_d = nc.dram_tensor("eps", [NLOC, TF], F32, kind="ExternalInput")
    # cq16[k] = [c0 chunk k (1024) | q chunk k (1024)] fp16
    cq_d = nc.dram_tensor("cq16", [NCHUNK, 2 * CHUNK], F16, kind="ExternalInput")
    obs_d = nc.dram_tensor("obs", [TF], F32, kind="ExternalInput")
    maskf_d = nc.dram_tensor("maskf", [TF], F32, kind="ExternalInput")
    out_d = nc.dram_tensor("out", [TF], F32, kind="ExternalOutput")

    ones_d = nc.dram_tensor("ones", [P], F32, kind="ExternalInput")
    ones16_d = nc.dram_tensor("ones16", [P], F16, kind="ExternalInput")
    sc_loc_d = nc.dram_tensor("sc_loc", [NLOC], F32)
    sc_all_d = nc.dram_tensor("sc_all", [N], F32, addr_space="Shared")
    # ws carries the TF weighted partials plus the local softmax
    # normalizer Z in slot TF — one AllReduce delivers both.
    ws_loc_d = nc.dram_tensor("ws_loc", [TF + 4], F32)
    ws_all_d = nc.dram_tensor("ws_all", [TF + 4], F32, addr_space="Shared")

    rg = [list(range(N_CORES))]

    with tile.TileContext(nc) as tc, ExitStack() as ctx:
        eps_ap = eps_d.ap()

        rowsq = ctx.enter_context(tc.tile_pool(name="rowsq", bufs=1))
        work = ctx.enter_context(tc.tile_pool(name="work", bufs=3))
        cache = ctx.enter_context(tc.tile_pool(name="cache", bufs=1))
        stat = ctx.enter_context(tc.tile_pool(name="stat", bufs=1))
        smal = ctx.enter_context(tc.tile_pool(name="smal", bufs=1))
        psum = ctx.enter_context(tc.tile_pool(name="psum", bufs=3, space="PSUM"))

        def pstile(name):
            return psum.tile([P, CHUNK], F32, tag="ps", bufs=3, name=name)

        # fp16 clipped-values cache: 64 tiles of [128, 1024] packed into
        # one persistent tile (128 KiB per partition)
        vcache = cache.tile([P, NBLK * NCHUNK * CHUNK], F16, tag="vc",
                            name="vcache")

        # fp16 ones row for the K=1 broadcast matmuls
        onesr16 = smal.tile([1, P], F16, tag="onesr16", name="onesr16")
        nc.sync.dma_start(
            out=onesr16[:], in_=ones16_d.ap().rearrange("(a n) -> a n", a=1)
        )
        # per-chunk [c0|q] row goes to partition 0 (matmul rhs needs base
        # partition 0); one 4 KB DMA per chunk

        # ---------------- pass A: local scores ----------------
        sd_cols = [
            stat.tile([P, NCHUNK], F32, tag=f"sd{b}", name=f"sd_cols{b}")
            for b in range(NBLK)
        ]
        s_loc = stat.tile([P, NBLK], F32, tag="sloc", name="s_loc")
        for k in range(NCHUNK):
            cqr = rowsq.tile([1, 2 * CHUNK], F16, tag="cqr", bufs=3,
                             name="cqr")
            nc.sync.dma_start(out=cqr[:], in_=cq_d.ap()[k:k + 1, :])
            # PE broadcast of c0/q rows (K=1 matmuls, fp16 in, f32 PSUM;
            # one matmul per 512-wide PSUM bank)
            psC = pstile("psC")
            psQ = pstile("psQ")
            for h in range(2):
                hs = slice(h * SUB, (h + 1) * SUB)
                nc.tensor.matmul(psC[:, hs], lhsT=onesr16[:],
                                 rhs=cqr[0:1, h * SUB:(h + 1) * SUB],
                                 start=True, stop=True)
                nc.tensor.matmul(psQ[:, hs], lhsT=onesr16[:],
                                 rhs=cqr[0:1, CHUNK + h * SUB:
                                             CHUNK + (h + 1) * SUB],
                                 start=True, stop=True)
            sub_on_dve = SUB_DVE_MOD and k % SUB_DVE_MOD == 0
            if not sub_on_dve:
                q16b = work.tile([P, CHUNK], F16, tag="q16b", bufs=2,
                                 name="q16b")
                nc.scalar.activation(out=q16b[:], in_=psQ[:], func=ACTF.Copy)
            sl = slice(k * CHUNK, (k + 1) * CHUNK)
            for b in range(NBLK):
                u_t = work.tile([P, CHUNK], F32, tag="u", bufs=3, name="u_t")
                nc.sync.dma_start(out=u_t[:], in_=eps_ap[b * P:(b + 1) * P, sl])
                u32 = work.tile([P, CHUNK], F32, tag="u32", bufs=2, name="u32")
                nc.vector.tensor_tensor(u32[:], u_t[:], psC[:], ALU.add)
                off = (k * NBLK + b) * CHUNK
                vsl = vcache[:, off:off + CHUNK]
                # fp32-in TS hits the DVE 2x mode (fp16-in measures 1.6x
                # slower on HW)
                nc.vector.tensor_scalar(
                    out=vsl, in0=u32[:], scalar1=inv_sig, scalar2=-inv_sig,
                    op0=ALU.min, op1=ALU.max,
                )
                d16 = work.tile([P, CHUNK], F16, tag="d16", bufs=3,
                                name="d16")
                if sub_on_dve:
                    # d = v - q with q read straight from PSUM
                    nc.vector.scalar_tensor_tensor(
                        out=d16[:], in0=psQ[:], scalar=-1.0, in1=vsl,
                        op0=ALU.mult, op1=ALU.add,
                    )
                else:
                    nc.gpsimd.tensor_tensor(d16[:], vsl, q16b[:],
                                            ALU.subtract)
                d2 = work.tile([P, CHUNK], F16, tag="d2", bufs=2, name="d2")
                nc.scalar.activation(
                    out=d2[:], in_=d16[:], func=ACTF.Square,
                    accum_out=sd_cols[b][:, k:k + 1],
                )
        # score finalize: s = cA*sum(d^2)
        for b in range(NBLK):
            sd_tot = smal.tile([P, 1], F32, tag="sdt", name="sd_tot")
            nc.vector.tensor_reduce(sd_tot[:], sd_cols[b][:], axis=AX.X, op=ALU.add)
            nc.vector.tensor_scalar_mul(s_loc[:, b:b + 1], sd_tot[:], cA)
        nc.sync.dma_start(
            out=sc_loc_d.ap().rearrange("(b p) -> p b", b=NBLK),
            in_=s_loc[:],
        )
        if stage >= 2:
            nc.gpsimd.collective_compute(
                "AllGather", ALU.bypass,
                ins=[sc_loc_d.ap()],
                outs=[sc_all_d.ap()],
                replica_groups=rg,
            )
        if stage <= 1:
            nc.sync.dma_start(
                out=out_d.ap()[0:NLOC].rearrange("(b p) -> p b", p=P),
                in_=s_loc[:],
            )

        # ---------------- softmax stats ----------------
        # weights are UN-normalized exp(); the global Z rides the
        # AllReduce (slot TF of ws) and division happens post-reduce.
        wt16 = None
        if stage >= 2:
            onesr = smal.tile([1, P], F32, tag="onesr", name="onesr")
            nc.sync.dma_start(
                out=onesr[:], in_=ones_d.ap().rearrange("(a n) -> a n", a=1)
            )
            onec = smal.tile([P, 1], F32, tag="onec", name="onec")
            nc.sync.dma_start(
                out=onec[:], in_=ones_d.ap().rearrange("(p a) -> p a", a=1)
            )
            s_all = smal.tile([1, N], F32, tag="sall", name="s_all")
            nc.sync.dma_start(
                out=s_all[:], in_=sc_all_d.ap().rearrange("(a n) -> a n", a=1)
            )
            pack = smal.tile([1, 2], F32, tag="pack", name="pack")
            negmean = smal.tile([1, 1], F32, tag="negmean", name="negmean")
            nc.vector.tensor_reduce(negmean[:], s_all[:], axis=AX.X, op=ALU.add)
            nc.vector.tensor_scalar_mul(negmean[:], negmean[:], -1.0 / N)
            js = smal.tile([1, N], F16, tag="js", name="js")
            ssq = smal.tile([1, 1], F32, tag="ssq", name="ssq")
            nc.scalar.activation(
                out=js[:], in_=s_all[:], func=ACTF.Square, bias=negmean[:],
                accum_out=ssq[:],
            )
            # std = max(sqrt(ssq/(N-1)), 1e-4); pack0 = 1/(std*TEMP)
            std = smal.tile([1, 1], F32, tag="std", name="std")
            nc.scalar.activation(
                out=std[:], in_=ssq[:], func=ACTF.Sqrt, scale=1.0 / (N - 1)
            )
            stdT = smal.tile([1, 1], F32, tag="stdT", name="stdT")
            nc.vector.tensor_scalar(
                out=stdT[:], in0=std[:], scalar1=1e-4, scalar2=TEMP,
                op0=ALU.max, op1=ALU.mult,
            )
            nc.vector.reciprocal(pack[:, 0:1], stdT[:])
            mx = smal.tile([1, 1], F32, tag="mx", name="mx")
            nc.vector.tensor_reduce(mx[:], s_all[:], axis=AX.X, op=ALU.max)
            # shifted logit: (s - mx)*inv10 (mean cancels in the shift, and
            # the un-normalized exp is safe: max exponent is exactly 0)
            nmx = smal.tile([1, 1], F32, tag="nmx", name="nmx")
            nc.vector.tensor_scalar_mul(nmx[:], mx[:], -1.0)
            nc.vector.tensor_tensor(pack[:, 1:2], nmx[:], pack[:, 0:1], ALU.mult)
            # PE-broadcast (inv10, bg) to all 128 partitions
            bps = pstile("bps")
            nc.tensor.matmul(bps[:, 0:2], lhsT=onesr[:], rhs=pack[:],
                             start=True, stop=True)
            scal = smal.tile([P, 2], F32, tag="scal", name="scal")
            nc.vector.tensor_copy(scal[:], bps[:, 0:2])

            # warm the PE p-state before pass B: back-to-back [1,512] dummy
            # matmuls gated on the post-stats scal tile keep the PE busy
            # >3us so the real fp16 matmuls run at 2.4 GHz, not 1.2
            jl = smal.tile([P, 1], F16, tag="jl", name="jl")
            nc.scalar.copy(jl[:], scal[:, 0:1])
            for w in range(10):
                wmm = pstile("wmm")
                nc.tensor.matmul(wmm[0:1, 0:SUB], lhsT=jl[:],
                                 rhs=vcache[:, 0:SUB], start=True, stop=True)

            e_loc = smal.tile([P, NBLK], F32, tag="eloc", name="e_loc")
            nc.scalar.activation(
                out=e_loc[:], in_=s_loc[:], func=ACTF.Exp,
                scale=scal[:, 0:1], bias=scal[:, 1:2],
            )
            wt16 = stat.tile([P, NBLK], F16, tag="wt16", name="wt16")
            zloc = smal.tile([P, 1], F32, tag="zloc", name="zloc")
            nc.scalar.activation(
                out=wt16[:], in_=e_loc[:], func=ACTF.Copy, accum_out=zloc[:]
            )
            # local Z -> ws_loc[TF] so the AllReduce sums it globally
            zpt = pstile("zpt")
            zps = zpt[0:1, 0:1]
            nc.tensor.matmul(zps, lhsT=zloc[:], rhs=onec[:], start=True,
                             stop=True)
            ztot = smal.tile([1, 1], F32, tag="ztot", name="ztot")
            nc.vector.tensor_copy(ztot[:], zps)
            nc.sync.dma_start(
                out=ws_loc_d.ap()[TF:TF + 1].rearrange("(a n) -> a n", a=1),
                in_=ztot[:],
            )
            if stage <= 2:
                nc.sync.dma_start(
                    out=out_d.ap()[0:NLOC].rearrange("(b p) -> p b", p=P),
                    in_=e_loc[:],
                )

        # ---------------- pass B: weighted sum on PE from SBUF cache ----
        if stage >= 3:
            # two 512-wide PSUM rows (= one 1024 chunk) per bounce tile:
            # halves the copy and writeback-DMA count
            for k in range(NCHUNK):
                wrow = pstile("wrow")
                for half in range(2):
                    for b in range(NBLK):
                        off = (k * NBLK + b) * CHUNK + half * SUB
                        nc.tensor.matmul(
                            wrow[0:1, half * SUB:(half + 1) * SUB],
                            lhsT=wt16[:, b:b + 1],
                            rhs=vcache[:, off:off + SUB],
                            start=(b == 0), stop=(b == NBLK - 1),
                        )
                wsb = work.tile([1, CHUNK], F32, tag="wsb", bufs=3, name="wsb")
                if k % 2 == 0:
                    nc.vector.tensor_copy(wsb[:], wrow[0:1, :])
                else:
                    nc.scalar.copy(wsb[:], wrow[0:1, :])
                nc.sync.dma_start(
                    out=ws_loc_d.ap()[k * CHUNK:(k + 1) * CHUNK]
                    .rearrange("(a n) -> a n", a=1),
                    in_=wsb[:],
                )
            if stage <= 3:
                o3 = stat.tile([P, TF // P], F32, tag="o3", name="o3")
                nc.sync.dma_start(
                    out=o3[:],
                    in_=ws_loc_d.ap()[0:TF].rearrange("(p c) -> p c", p=P),
                )
                nc.sync.dma_start(
                    out=out_d.ap().rearrange("(p c) -> p c", p=P), in_=o3[:]
                )

        # ---------------- AllReduce + final combine ----------------
        if stage >= 4:
            rowmaj0 = lambda d: d.ap()[0:TF].rearrange("(p c) -> p c", p=P)
            obs_t = stat.tile([P, TF // P], F32, tag="obsf", name="obs_t")
            nc.sync.dma_start(out=obs_t[:], in_=rowmaj0(obs_d))
            m_t = stat.tile([P, TF // P], F32, tag="mf", name="m_t")
            nc.sync.dma_start(out=m_t[:], in_=rowmaj0(maskf_d))
            nc.gpsimd.collective_compute(
                "AllReduce", ALU.add,
                ins=[ws_loc_d.ap()], outs=[ws_all_d.ap()], replica_groups=rg,
            )
            w_t = stat.tile([P, TF // P], F32, tag="wfin", name="w_t")
            nc.sync.dma_start(out=w_t[:], in_=rowmaj0(ws_all_d))
            zg = smal.tile([1, 1], F32, tag="zg", name="zg")
            nc.sync.dma_start(
                out=zg[:],
                in_=ws_all_d.ap()[TF:TF + 1].rearrange("(a n) -> a n", a=1),
            )
            rzg = smal.tile([1, 1], F32, tag="rzg", name="rzg")
            nc.vector.reciprocal(rzg[:], zg[:])
            qfin = smal.tile([1, 1], F32, tag="qfin", name="qfin")
            nc.vector.tensor_scalar_mul(qfin[:], rzg[:], float(c1 * sigma_i))
            qps = pstile("qps")
            nc.tensor.matmul(qps[:, 0:1], lhsT=onesr[:], rhs=qfin[:],
                             start=True, stop=True)
            qb = smal.tile([P, 1], F32, tag="qb", name="qb")
            nc.vector.tensor_copy(qb[:], qps[:, 0:1])
            t1 = stat.tile([P, TF // P], F32, tag="t1", name="t1")
            nc.vector.tensor_single_scalar(
                out=t1[:], in_=w_t[:], scalar=qb[:], op=ALU.mult
            )
            # in-place: obs_t <- m*(obs - t1); t1 <- t1 + obs_t
            nc.vector.tensor_tensor(obs_t[:], obs_t[:], t1[:], ALU.subtract)
            nc.vector.tensor_tensor(obs_t[:], obs_t[:], m_t[:], ALU.mult)
            nc.vector.tensor_tensor(t1[:], t1[:], obs_t[:], ALU.add)
            nc.sync.dma_start(out=rowmaj0(out_d), in_=t1[:])

    nc.compile()
    return nc


_CACHE: dict = {}
TRACE = False
STAGE = 4
LAST_RESULTS = None


def kernel(Xbar_i, observed_data, time_points, mask, eps, deg_a, deg_b, i):
    global LAST_RESULTS
    i = int(i)
    sigma_i, c1 = _schedule_scalars(i)
    key = ("v6", i, STAGE, SUB_DVE_MOD)
    if key not in _CACHE:
        _CACHE[key] = _build(float(sigma_i), float(c1), stage=STAGE)
    nc = _CACHE[key]

    inv_sig = np.float32(1.0) / sigma_i
    Xb = np.asarray(Xbar_i, np.float32)
    obs = np.asarray(observed_data, np.float32)
    msk = np.asarray(mask, bool)
    tp = np.asarray(time_points, np.float32)
    da = np.asarray(deg_a, np.float32)
    db = np.asarray(deg_b, np.float32)
    epsf = np.asarray(eps, np.float32)

    pred = da[None, :] + db[None, :] * tp[:, None]
    c0 = (Xb * inv_sig).astype(np.float32)
    c0 = np.where(msk, np.float32(SAT), c0).reshape(-1)
    qp = (pred * inv_sig).astype(np.float32)
    qp = np.where(msk, inv_sig, qp).reshape(-1)
    # cq16[k] = [c0 chunk | q chunk] fp16
    cq16 = np.concatenate(
        [c0.reshape(NCHUNK, CHUNK), qp.reshape(NCHUNK, CHUNK)], axis=1
    ).astype(np.float16)
    obsf = obs.reshape(-1)
    maskf = msk.astype(np.float32).reshape(-1)

    in_maps = []
    for c in range(N_CORES):
        shard = np.ascontiguousarray(
            epsf[c * NLOC:(c + 1) * NLOC].reshape(NLOC, TF)
        )
        in_maps.append(
            {"eps": shard, "cq16": cq16, "obs": obsf, "maskf": maskf,
             "ones": np.ones(P, np.float32),
             "ones16": np.ones(P, np.float16)}
        )
    kr = run_bass_kernel_spmd(nc, in_maps, list(range(N_CORES)), trace=TRACE)
    LAST_RESULTS = kr
    return kr.results[0]["out"].reshape(T, F).astype(np.float32)
